# revision 2
# baseline (speedup 1.0000x reference)
"""Causal attention kernel for 8 TRN2 NeuronCores.

Problem: B=4, S=4096, D=1024 single-head causal attention with QKV projection.
  q/k/v = x @ W{q,k,v}.T ; out = softmax(tril(q k^T)/sqrt(D)) @ v

Sharding: core c -> batch b = c//2, parity p = c%2. Each core computes the
output rows of the 16 seq blocks (128 rows each) of batch b with block index
parity p ("striped" sequence parallelism -> balanced causal work). Every core
projects k/v for the full batch (weights replicated), q only for its own rows.

The SPMD program is identical on all cores; per-core differences (which rows,
causal mask parity) are pushed into the *data*: the host permutes x's row
blocks to [own | partner] order and builds a small parity-dependent band mask.
"""

import sys

import numpy as np

sys.path.insert(0, "/opt/trn_rl_repo")

import concourse.bass as bass  # noqa: E402
import concourse.mybir as mybir  # noqa: E402
import concourse.tile as tile  # noqa: E402
from concourse import bacc  # noqa: E402
from concourse.bass_utils import run_bass_kernel_spmd  # noqa: E402

import ml_dtypes  # noqa: E402

B, S, D = 4, 4096, 1024
P = 128
NB = S // P          # 32 seq blocks per batch
NLB = NB // 2        # 16 own blocks per core
NG = 4               # attention q-groups of 512 rows (4 local blocks each)
SCALE = 1.0 / 32.0   # 1/sqrt(D)

BF16 = mybir.dt.bfloat16
F32 = mybir.dt.float32

_built = {}


def _build_nc():
    nc = bacc.Bacc("TRN2", target_bir_lowering=False, debug=False, num_devices=8)

    xt = nc.declare_dram_parameter("xt", [D, S], BF16, isOutput=False)
    wqt = nc.declare_dram_parameter("wqt", [D, D], BF16, isOutput=False)
    wkt = nc.declare_dram_parameter("wkt", [D, D], BF16, isOutput=False)
    wvt = nc.declare_dram_parameter("wvt", [D, D], BF16, isOutput=False)
    maskp = nc.declare_dram_parameter("mask", [P, 8 * 512], BF16, isOutput=False)
    y = nc.declare_dram_parameter("y", [S // 2, D], F32, isOutput=True)
    vdram = nc.dram_tensor("vdram", [NB, P, D], BF16)

    xt3 = xt.ap().rearrange("(po pi) s -> pi po s", pi=P)       # [128, 8, 4096]
    wqt3 = wqt.ap().rearrange("(po pi) e -> pi po e", pi=P)
    wkt3 = wkt.ap().rearrange("(po pi) e -> pi po e", pi=P)
    wvt3 = wvt.ap().rearrange("(po pi) e -> pi po e", pi=P)
    mask3 = maskp.ap().rearrange("p (r q) -> p r q", r=8)       # [128, 8, 512]
    y3 = y.ap().rearrange("(nb pi) e -> nb pi e", pi=P)         # [16, 128, 1024]

    with tile.TileContext(nc) as tc:
        with (
            tc.tile_pool(name="consts", bufs=1) as consts,
            tc.tile_pool(name="wp", bufs=1) as wp,
            tc.tile_pool(name="xtp", bufs=2) as xtp,
            tc.tile_pool(name="qtp", bufs=1) as qtp,
            tc.tile_pool(name="ktp", bufs=1) as ktp,
            tc.tile_pool(name="vstg", bufs=2) as vstg,
            tc.tile_pool(name="strip", bufs=32) as strip,
            tc.tile_pool(name="vload", bufs=3) as vload,
            tc.tile_pool(name="linvp", bufs=2) as linvp,
            tc.tile_pool(name="ctxs", bufs=3) as ctxs,
            tc.tile_pool(name="psum", bufs=8, space="PSUM") as psum,
        ):
            mask_sb = consts.tile([P, 8, 512], BF16)
            nc.sync.dma_start(mask_sb[:], mask3)
            ones_sb = consts.tile([P, P], BF16)
            nc.gpsimd.memset(ones_sb[:], 1.0)

            qt_sb = qtp.tile([P, 8, S // 2], BF16)   # q^T: [e, own 2048 rows]
            kt_sb = ktp.tile([P, 8, S], BF16)        # k^T: [e, all 4096 rows]

            # ---- Projection phase: one pass per weight (V, K, Q) ----
            # V pass: v = x @ Wv.T in natural [s, e] layout, spilled to DRAM.
            wv_sb = wp.tile([P, 8, D], BF16, tag="w")
            nc.sync.dma_start(wv_sb[:], wvt3)
            for c in range(8):
                xt_t = xtp.tile([P, 8, 512], BF16, tag="xt")
                nc.sync.dma_start(xt_t[:], xt3[:, :, c * 512:(c + 1) * 512])
                for sb in range(4):
                    vst = vstg.tile([P, D], BF16, tag="vst")
                    for eh in range(2):
                        ps = psum.tile([P, 512], F32, tag="bank")
                        for dc in range(8):
                            nc.tensor.matmul(
                                ps[:],
                                lhsT=xt_t[:, dc, sb * P:(sb + 1) * P],
                                rhs=wv_sb[:, dc, eh * 512:(eh + 1) * 512],
                                start=(dc == 0),
                                stop=(dc == 7),
                            )
                        nc.vector.tensor_copy(out=vst[:, eh * 512:(eh + 1) * 512], in_=ps[:])
                    nc.sync.dma_start(vdram.ap()[c * 4 + sb], vst[:])

            # K pass: k^T = (x @ Wk.T)^T = Wk x^T in [e, s] layout, kept in SBUF.
            wk_sb = wp.tile([P, 8, D], BF16, tag="w")
            nc.sync.dma_start(wk_sb[:], wkt3)
            for c in range(8):
                xt_t = xtp.tile([P, 8, 512], BF16, tag="xt")
                nc.sync.dma_start(xt_t[:], xt3[:, :, c * 512:(c + 1) * 512])
                for ec in range(8):
                    ps = psum.tile([P, 512], F32, tag="bank")
                    for dc in range(8):
                        nc.tensor.matmul(
                            ps[:],
                            lhsT=wk_sb[:, dc, ec * P:(ec + 1) * P],
                            rhs=xt_t[:, dc, :],
                            start=(dc == 0),
                            stop=(dc == 7),
                        )
                    nc.vector.tensor_copy(out=kt_sb[:, ec, c * 512:(c + 1) * 512], in_=ps[:])

            # Q pass: q^T for own rows only (first 2048 permuted columns).
            wq_sb = wp.tile([P, 8, D], BF16, tag="w")
            nc.sync.dma_start(wq_sb[:], wqt3)
            for c in range(4):
                xt_t = xtp.tile([P, 8, 512], BF16, tag="xt")
                nc.sync.dma_start(xt_t[:], xt3[:, :, c * 512:(c + 1) * 512])
                for ec in range(8):
                    ps = psum.tile([P, 512], F32, tag="bank")
                    for dc in range(8):
                        nc.tensor.matmul(
                            ps[:],
                            lhsT=wq_sb[:, dc, ec * P:(ec + 1) * P],
                            rhs=xt_t[:, dc, :],
                            start=(dc == 0),
                            stop=(dc == 7),
                        )
                    nc.vector.tensor_copy(out=qt_sb[:, ec, c * 512:(c + 1) * 512], in_=ps[:])

            # ---- Attention phase ----
            for g in range(NG):
                n_own = 4 * g + 4
                # kb list: (is_partner, block idx) in permuted order
                kbs = [(0, o) for o in range(n_own)] + [(1, o) for o in range(n_own)]
                nkb = len(kbs)
                qcols = slice(g * 512, (g + 1) * 512)

                lrep_ps = psum.tile([P, 512], F32, tag="bank")
                pts = []
                for kb_idx, (part, o) in enumerate(kbs):
                    kcol = part * (S // 2) + o * P
                    st_ps = psum.tile([P, 512], F32, tag="bank")
                    for ec in range(8):
                        nc.tensor.matmul(
                            st_ps[:],
                            lhsT=kt_sb[:, ec, kcol:kcol + P],
                            rhs=qt_sb[:, ec, qcols],
                            start=(ec == 0),
                            stop=(ec == 7),
                        )
                    pt = strip.tile([P, 512], BF16, tag="pt")
                    nc.scalar.activation(
                        pt[:], st_ps[:], mybir.ActivationFunctionType.Exp, scale=SCALE
                    )
                    r = o - 4 * g + 4 * part  # band index 0..7, or negative if below band
                    if o >= 4 * g:
                        nc.vector.tensor_mul(out=pt[:], in0=pt[:], in1=mask_sb[:, r, :])
                    # denominator: column sums replicated across partitions
                    nc.tensor.matmul(
                        lrep_ps[:],
                        lhsT=ones_sb[:],
                        rhs=pt[:],
                        start=(kb_idx == 0),
                        stop=(kb_idx == nkb - 1),
                    )
                    pts.append(pt)

                linv = linvp.tile([P, 512], F32, tag="linv")
                nc.vector.reciprocal(linv[:], lrep_ps[:])
                for pt in pts:
                    nc.vector.tensor_mul(out=pt[:], in0=pt[:], in1=linv[:])

                # PV pass: ctx[qb][eh] accumulates over all key blocks
                ctx_ps = [[psum.tile([P, 512], F32, tag="bank", name=f"ctx_{g}_{qb}_{eh}")
                           for eh in range(2)] for qb in range(4)]
                for kb_idx, (part, o) in enumerate(kbs):
                    vb = part * NLB + o
                    vt = vload.tile([P, D], BF16, tag="vt")
                    nc.sync.dma_start(vt[:], vdram.ap()[vb])
                    for qb in range(4):
                        for eh in range(2):
                            nc.tensor.matmul(
                                ctx_ps[qb][eh][:],
                                lhsT=pts[kb_idx][:, qb * P:(qb + 1) * P],
                                rhs=vt[:, eh * 512:(eh + 1) * 512],
                                start=(kb_idx == 0),
                                stop=(kb_idx == nkb - 1),
                            )

                for qb in range(4):
                    for eh in range(2):
                        cs = ctxs.tile([P, 512], F32, tag="cs")
                        nc.scalar.copy(cs[:], ctx_ps[qb][eh][:])
                        nc.sync.dma_start(
                            y3[4 * g + qb, :, eh * 512:(eh + 1) * 512], cs[:]
                        )

    nc.compile()
    return nc


def _host_inputs(x, Wq, Wk, Wv):
    """Build per-core input maps. x: [B,S,D] f32; W*: [D,D] f32."""
    bf = ml_dtypes.bfloat16
    wqt = np.ascontiguousarray(Wq.T).astype(bf)
    wkt = np.ascontiguousarray(Wk.T).astype(bf)
    wvt = np.ascontiguousarray(Wv.T).astype(bf)

    in_maps = []
    for c in range(8):
        b, p = c // 2, c % 2
        own = [2 * j + p for j in range(NLB)]
        partner = [2 * j + (1 - p) for j in range(NLB)]
        xb = x[b].reshape(NB, P, D)[own + partner].reshape(S, D)
        xt = np.ascontiguousarray(xb.T).astype(bf)  # [D, S]

        # band mask [128 kj, 8 r, 512 qi]: r<4 own key blocks, r>=4 partner
        kj = np.arange(P)[:, None]
        qi = np.arange(512)[None, :]
        j2 = qi // P
        qrow = qi % P
        qpos = (2 * j2 + p) * P + qrow
        mask = np.zeros((P, 8, 512), np.float32)
        for r in range(8):
            kblk = 2 * r + p if r < 4 else 2 * (r - 4) + (1 - p)
            kpos = kblk * P + kj
            mask[:, r, :] = (kpos <= qpos).astype(np.float32)
        in_maps.append({
            "xt": xt,
            "wqt": wqt,
            "wkt": wkt,
            "wvt": wvt,
            "mask": mask.reshape(P, 8 * 512).astype(bf),
        })
    return in_maps


def kernel(**inputs):
    x = np.asarray(inputs["inputs"], np.float32)
    Wq = np.asarray(inputs["Wq"], np.float32)
    Wk = np.asarray(inputs["Wk"], np.float32)
    Wv = np.asarray(inputs["Wv"], np.float32)

    if "nc" not in _built:
        _built["nc"] = _build_nc()
    nc = _built["nc"]

    in_maps = _host_inputs(x, Wq, Wk, Wv)
    res = run_bass_kernel_spmd(nc, in_maps, core_ids=list(range(8)))

    out = np.empty((B, S, D), np.float32)
    for c in range(8):
        b, p = c // 2, c % 2
        yc = res.results[c]["y"].reshape(NLB, P, D)
        ob = out[b].reshape(NB, P, D)
        for j in range(NLB):
            ob[2 * j + p] = yc[j]
    return out
